# revision 10
# baseline (speedup 1.0000x reference)
"""GCN encoder (2-layer GCNConv) as a Bass/Tile kernel on 8 Trainium2 NeuronCores.

Strategy (matches the sharding hint):
  - Nodes row-partitioned across 8 cores (6250 rows each); weights replicated.
  - Symmetric normalization factorized: z = D^-1/2 (A+I) D^-1/2 (x W) + b
    =>  u = dinv * (x W);  agg[d] = u[d] + sum_{e:dst=d} u[src_e];
        z = dinv * agg + b
    so no per-edge norm gather is needed.
  - Per layer: local matmul -> row scale -> AllGather(u) -> per-core gather of
    source rows (dma_gather) -> segment-sum via tensor-engine matmuls with
    compile-time-structured 0/1 selection matrices generated on DVE
    (is_equal against an iota) -> scale/bias/relu -> output rows.
  - Edges are bucketed host-side by (dst window of 128, src half) and padded to
    128-slot tiles; padded slots gather row 0 and have an all-zero selection
    column, so they contribute nothing.  int16 gather indices require the
    src-half split (indices < 32768).
"""

import math
import os
import sys

import numpy as np

sys.path.insert(0, "/opt/trn_rl_repo")

import ml_dtypes

BF16 = ml_dtypes.bfloat16


class Cfg:
    def __init__(self, N, E, IN=512, HID=256, OUT=128, P=8, half=None):
        self.N, self.E, self.IN, self.HID, self.OUT, self.P = N, E, IN, HID, OUT, P
        self.NC = N // P                      # nodes per core
        self.WS = 128                         # dst window size
        self.NW = math.ceil(self.NC / self.WS)  # windows per core
        # src-half split point (int16 gather indices must stay < 32768)
        if half is None:
            half = N if N <= 32767 else (N + 1) // 2
        self.HALF = half
        assert self.HALF <= 32767 and N - self.HALF <= 32767


FULL = Cfg(N=50000, E=800000)


def _prepare(cfg, x, edge_index, W1, b1, W2, b2):
    """Host-side graph preprocessing -> per-core input maps + program params."""
    N, P, NC, WS, NW, HALF = cfg.N, cfg.P, cfg.NC, cfg.WS, cfg.NW, cfg.HALF
    src = np.asarray(edge_index[0], dtype=np.int64)
    dst = np.asarray(edge_index[1], dtype=np.int64)

    deg = np.bincount(dst, minlength=N).astype(np.float64) + 1.0  # + self loop
    dinv = (1.0 / np.sqrt(deg)).astype(np.float32)

    # group id: ((core, window), src-half) ; groups contiguous after sort
    win_id = (dst // NC) * NW + (dst % NC) // WS
    half = (src >= HALF).astype(np.int64)
    comp = win_id * 2 + half
    order = np.argsort(comp, kind="stable")
    s_s, d_s, c_s = src[order], dst[order], comp[order]
    counts = np.bincount(c_s, minlength=P * NW * 2).reshape(P, NW, 2)

    # shared tile counts per (window, half): max over cores
    T = np.ceil(counts.max(axis=0) / 128).astype(np.int64)  # [NW, 2]
    tiles_total = int(T.sum())
    slots_total = tiles_total * 128

    starts = np.zeros(P * NW * 2 + 1, dtype=np.int64)
    np.cumsum(counts.reshape(-1), out=starts[1:])

    dinv_pad = np.concatenate(
        [dinv, np.ones(NW * WS * P - N, dtype=np.float32)])

    in_maps = []
    for c in range(P):
        # pad slots get idx=-1: the gather ucode trims trailing negative
        # indices at runtime, skipping descriptor generation for padding
        idx_arr = np.full(slots_total, -1, dtype=np.int16)
        aco_arr = np.full(slots_total, -1, dtype=np.float32)  # cast to bf16 below
        off = 0
        for w in range(NW):
            for h in range(2):
                g = (c * NW + w) * 2 + h
                n = counts[c, w, h]
                sl = slice(starts[g], starts[g] + n)
                idx_arr[off:off + n] = (s_s[sl] - h * HALF).astype(np.int16)
                aco_arr[off:off + n] = (d_s[sl] - c * NC - w * WS).astype(np.float32)
                off += 128 * int(T[w, h])
        assert off == slots_total

        dloc = np.concatenate(
            [dinv[c * NC:(c + 1) * NC],
             np.ones(NW * WS - NC, dtype=np.float32)])

        m = {
            "cnt": np.ascontiguousarray(
                counts[c].reshape(1, NW * 2).astype(np.int32)),
            "xT": np.ascontiguousarray(
                np.asarray(x[c * NC:(c + 1) * NC], np.float32).astype(BF16).T),
            "w1": np.ascontiguousarray(
                np.asarray(W1, np.float32).astype(BF16)
                .reshape(cfg.IN // 128, 128, cfg.HID).transpose(1, 0, 2)),
            "w2": np.ascontiguousarray(
                np.asarray(W2, np.float32).astype(BF16)
                .reshape(cfg.HID // 128, 128, cfg.OUT).transpose(1, 0, 2)),
            "dinvc": np.ascontiguousarray(dloc.reshape(NW, WS).T),
            "idx": np.ascontiguousarray(np.tile(idx_arr.reshape(-1, 16).T, (8, 1))),
            "acol": np.ascontiguousarray(aco_arr.reshape(-1, 128).T.astype(BF16)),
            "ident": np.eye(128, dtype=BF16),
        }
        b1nz = bool(np.any(np.asarray(b1)))
        b2nz = bool(np.any(np.asarray(b2)))
        if b1nz:
            m["b1bc"] = np.ascontiguousarray(
                np.broadcast_to(np.asarray(b1, np.float32), (128, cfg.HID)))
        if b2nz:
            m["b2bc"] = np.ascontiguousarray(
                np.broadcast_to(np.asarray(b2, np.float32), (128, cfg.OUT)))
        in_maps.append(m)

    return in_maps, T, b1nz, b2nz


def build_program(cfg, T, b1nz, b2nz):
    import concourse.bass as bass
    import concourse.bacc as bacc
    import concourse.mybir as mybir
    from concourse import tile

    N, P, NC, WS, NW = cfg.N, cfg.P, cfg.NC, cfg.WS, cfg.NW
    IN, HID, OUT = cfg.IN, cfg.HID, cfg.OUT
    NCI, NCH = IN // 128, HID // 128
    tiles_total = int(T.sum())
    slots_total = tiles_total * 128
    f32, bf16, i16 = mybir.dt.float32, mybir.dt.bfloat16, mybir.dt.int16
    AF = mybir.ActivationFunctionType

    nc = bacc.Bacc("TRN2", target_bir_lowering=False, debug=False,
                   num_devices=cfg.P)
    xT_p = nc.dram_tensor("xT", [IN, NC], bf16, kind="ExternalInput")
    w1_p = nc.dram_tensor("w1", [128, NCI, HID], bf16, kind="ExternalInput")
    w2_p = nc.dram_tensor("w2", [128, NCH, OUT], bf16, kind="ExternalInput")
    dinv_p = nc.dram_tensor("dinvc", [WS, NW], f32, kind="ExternalInput")
    idx_p = nc.dram_tensor("idx", [128, slots_total // 16], i16, kind="ExternalInput")
    cnt_p = nc.dram_tensor("cnt", [1, NW * 2], mybir.dt.int32, kind="ExternalInput")
    acol_p = nc.dram_tensor("acol", [128, tiles_total], bf16, kind="ExternalInput")
    id_p = nc.dram_tensor("ident", [128, 128], bf16, kind="ExternalInput")
    b1_p = (nc.dram_tensor("b1bc", [128, HID], f32, kind="ExternalInput")
            if b1nz else None)
    b2_p = (nc.dram_tensor("b2bc", [128, OUT], f32, kind="ExternalInput")
            if b2nz else None)
    out_p = nc.dram_tensor("out", [NC, OUT], f32, kind="ExternalOutput")

    u1d = nc.dram_tensor("u1d", [NC, HID], bf16)
    u2d = nc.dram_tensor("u2d", [NC, OUT], bf16)
    U1 = nc.dram_tensor("U1", [N, HID], bf16)
    U2 = nc.dram_tensor("U2", [N, OUT], bf16)
    rg = [list(range(P))]

    with tile.TileContext(nc) as tc:
        with (
            tc.tile_pool(name="res", bufs=1) as res,
            tc.tile_pool(name="work", bufs=4) as work,
            tc.tile_pool(name="gath", bufs=4) as gath,
            tc.tile_pool(name="psum", bufs=2, space="PSUM") as psum,
        ):
            # ---- resident loads ----
            xTs = res.tile([128, NCI, NC], bf16)
            for ci in range(NCI):
                nc.sync.dma_start(xTs[:, ci, :], xT_p[ci * 128:(ci + 1) * 128, :])
            w1s = res.tile([128, NCI, HID], bf16)
            nc.sync.dma_start(w1s[:], w1_p[:])
            w2s = res.tile([128, NCH, OUT], bf16)
            nc.sync.dma_start(w2s[:], w2_p[:])
            dinvs = res.tile([WS, NW], f32)
            nc.sync.dma_start(dinvs[:], dinv_p[:])
            idxs = res.tile([128, slots_total // 16], i16)
            nc.sync.dma_start(idxs[:], idx_p[:])
            cnts = res.tile([1, NW * 2], mybir.dt.int32)
            nc.sync.dma_start(cnts[:], cnt_p[:])
            acols = res.tile([128, tiles_total], bf16)
            nc.sync.dma_start(acols[:], acol_p[:])
            ident = res.tile([128, 128], bf16)
            nc.sync.dma_start(ident[:], id_p[:])
            iot = res.tile([128, 128], bf16)
            nc.gpsimd.iota(iot[:], pattern=[[1, 128]], base=0,
                           channel_multiplier=0,
                           allow_small_or_imprecise_dtypes=True)
            b1bc = None
            if b1nz:
                b1bc = res.tile([128, HID], f32)
                nc.sync.dma_start(b1bc[:], b1_p[:])
            b2bc = None
            if b2nz:
                b2bc = res.tile([128, OUT], f32)
                nc.sync.dma_start(b2bc[:], b2_p[:])

            # manually-rotated gather staging buffer, zeroed once up front:
            # runtime-trimmed pad slots are never written by the DMA, so the
            # bytes they alias must stay finite (0 * NaN would poison PSUM)
            gmax = int(T.max())
            GB = 4
            gbuf = res.tile([128, GB * gmax * HID], bf16)
            nc.gpsimd.memset(gbuf[:], 0.0)

            u1res = res.tile([128, NW, HID], bf16)
            u2res = res.tile([128, NW, OUT], bf16)
            h1T = res.tile([128, NCH, NC], bf16)
            if NC % WS:
                # tail rows of the last window feed the self-loop matmul as
                # rhs; zero them so uninitialized SBUF can't inject NaNs
                nc.gpsimd.memset(u1res[:, NW - 1, :], 0.0)
                nc.gpsimd.memset(u2res[:, NW - 1, :], 0.0)

            def nsz(j):
                return min(128, NC - j * WS)

            MAXP = int(os.environ.get("GCN_MAX_PHASE", "9"))

            def emit_debug_out(src_bf16_ap, w, n):
                # convert [n, OUT] bf16 -> f32, dump into out rows of window w
                dt = work.tile([128, OUT], f32, tag="dbg")
                nc.scalar.activation(dt[:n, :], src_bf16_ap, AF.Copy)
                nc.sync.dma_start(out_p[w * WS:w * WS + n, :], dt[:n, :])

            # ---- phase A: t1 = x @ W1 ; u1 = dinv * t1 ----
            for j in range(NW):
                n = nsz(j)
                jsl = slice(j * WS, j * WS + n)
                pt = psum.tile([128, HID], f32, tag="mm")
                for ci in range(NCI):
                    nc.tensor.matmul(pt[:n, :], xTs[:, ci, jsl],
                                     w1s[:, ci, :], start=(ci == 0),
                                     stop=(ci == NCI - 1))
                nc.scalar.activation(u1res[:n, j, :], pt[:n, :], AF.Copy,
                                     scale=dinvs[:n, j:j + 1])
                nc.sync.dma_start(u1d[jsl, :], u1res[:n, j, :])
                if MAXP == 1:
                    emit_debug_out(u1res[:n, j, :OUT], j, n)
            if MAXP <= 1:
                return nc

            # ---- AllGather u1 ----
            nc.gpsimd.collective_compute(
                "AllGather", mybir.AluOpType.bypass, replica_groups=rg,
                ins=[u1d[:]], outs=[U1[:]])
            if MAXP == 2:
                for j in range(NW):
                    n = nsz(j)
                    gt = work.tile([128, OUT], bf16, tag="dbg_g")
                    nc.sync.dma_start(gt[:n, :], U1[j * WS:j * WS + n, :OUT])
                    emit_debug_out(gt[:n, :], j, n)
                return nc

            # ---- generic aggregation layer ----
            gslot = [0]

            def agg_layer(U, F, ures, bbc, relu, emit_out):
                tile_idx = 0
                slot_off = 0
                for w in range(NW):
                    n = nsz(w)
                    pa = psum.tile([128, F], f32, tag="agg")
                    # self-loop term: ident.T @ u[w]
                    nc.tensor.matmul(pa[:n, :], ident[:, :n], ures[:, w, :],
                                     start=True, stop=False)
                    nmm = int(T[w, 0] + T[w, 1])
                    done = 0
                    for h in range(2):
                        t_wh = int(T[w, h])
                        if t_wh == 0:
                            continue
                        goff = (gslot[0] % GB) * gmax * HID
                        gslot[0] += 1
                        g = gbuf[:, goff:goff + t_wh * F].rearrange(
                            "p (t f) -> p t f", t=t_wh)
                        base = 0 if h == 0 else cfg.HALF
                        nval = nc.values_load(
                            cnts[:, (w * 2 + h):(w * 2 + h) + 1],
                            engines=[mybir.EngineType.Pool],
                            skip_runtime_bounds_check=True)
                        nc.gpsimd.dma_gather(
                            g[:], U[base:base + min(cfg.HALF, N - base), :],
                            idxs[:, slot_off // 16:
                                 (slot_off + 128 * t_wh) // 16],
                            num_idxs=128 * t_wh, num_idxs_reg=nval,
                            elem_size=F, single_packet=False)
                        slot_off += 128 * t_wh
                        for t in range(t_wh):
                            S = work.tile([128, 128], bf16, tag="S")
                            nc.vector.tensor_tensor(
                                S[:], iot[:],
                                acols[:, tile_idx:tile_idx + 1]
                                .broadcast_to((128, 128)),
                                op=mybir.AluOpType.is_equal)
                            tile_idx += 1
                            done += 1
                            nc.tensor.matmul(pa[:n, :], S[:, :n], g[:, t, :],
                                             start=False, stop=(done == nmm))
                    # z = dinv * agg (+ b) ; relu
                    if bbc is None:
                        zf = AF.Relu if relu else AF.Copy
                        zt = work.tile([128, F], f32 if emit_out else bf16,
                                       tag="zt%d" % F)
                        nc.scalar.activation(zt[:n, :], pa[:n, :], zf,
                                             scale=dinvs[:n, w:w + 1])
                    else:
                        v = work.tile([128, F], f32, tag="v%d" % F)
                        nc.scalar.activation(v[:n, :], pa[:n, :], AF.Copy,
                                             scale=dinvs[:n, w:w + 1])
                        zt = work.tile([128, F], f32 if emit_out else bf16,
                                       tag="zt%d" % F)
                        if relu:
                            vb = work.tile([128, F], f32, tag="vb%d" % F)
                            nc.vector.tensor_tensor(
                                vb[:n, :], v[:n, :], bbc[:n, :],
                                op=mybir.AluOpType.add)
                            nc.scalar.activation(zt[:n, :], vb[:n, :], AF.Relu)
                        else:
                            nc.vector.tensor_tensor(
                                zt[:n, :], v[:n, :], bbc[:n, :],
                                op=mybir.AluOpType.add)
                    yield w, n, zt

            # ---- phase C: layer-1 aggregation -> h1 -> h1T ----
            for w, n, zt in agg_layer(U1, HID, u1res, b1bc, True, False):
                wsl = slice(w * WS, w * WS + n)
                for ch in range(NCH):
                    ptr = psum.tile([128, 128], bf16, tag="tr")
                    nc.tensor.transpose(ptr[:, :n],
                                        zt[:n, ch * 128:(ch + 1) * 128],
                                        ident[:n, :n])
                    nc.scalar.activation(h1T[:, ch, wsl], ptr[:, :n], AF.Copy)
                if MAXP == 3:
                    emit_debug_out(zt[:n, :OUT], w, n)
            if MAXP <= 3:
                return nc

            # ---- phase D: t2 = h1 @ W2 ; u2 ----
            for j in range(NW):
                n = nsz(j)
                jsl = slice(j * WS, j * WS + n)
                pt = psum.tile([128, OUT], f32, tag="mm")
                for ch in range(NCH):
                    nc.tensor.matmul(pt[:n, :], h1T[:, ch, jsl],
                                     w2s[:, ch, :], start=(ch == 0),
                                     stop=(ch == NCH - 1))
                nc.scalar.activation(u2res[:n, j, :], pt[:n, :], AF.Copy,
                                     scale=dinvs[:n, j:j + 1])
                nc.sync.dma_start(u2d[jsl, :], u2res[:n, j, :])
                if MAXP == 4:
                    emit_debug_out(u2res[:n, j, :], j, n)
            if MAXP <= 4:
                return nc

            # ---- AllGather u2 ----
            nc.gpsimd.collective_compute(
                "AllGather", mybir.AluOpType.bypass, replica_groups=rg,
                ins=[u2d[:]], outs=[U2[:]])

            # ---- phase F: layer-2 aggregation -> out ----
            for w, n, zt in agg_layer(U2, OUT, u2res, b2bc, False, True):
                wsl = slice(w * WS, w * WS + n)
                nc.sync.dma_start(out_p[wsl, :], zt[:n, :])

    return nc


def run(cfg, inputs, sim=False, trace=False):
    from concourse.bass_utils import run_bass_kernel_spmd

    in_maps, T, b1nz, b2nz = _prepare(
        cfg, inputs["x"], inputs["edge_index"], inputs["W1"], inputs["b1"],
        inputs["W2"], inputs["b2"])
    nc = build_program(cfg, T, b1nz, b2nz)
    nc.finalize()
    core_ids = list(range(cfg.P))
    if sim:
        from concourse import bass_interp
        ms = bass_interp.MultiCoreSim(nc, cfg.P)
        for c in core_ids:
            for k, v in in_maps[c].items():
                ms.cores[c].tensor(k)[:] = v
        ms.simulate()
        outs = [np.array(ms.cores[c].tensor("out")) for c in core_ids]
        return np.concatenate(outs, axis=0), None
    res = run_bass_kernel_spmd(nc, in_maps, core_ids, trace=trace)
    outs = [np.asarray(res.results[c]["out"]) for c in core_ids]
    return np.concatenate(outs, axis=0), res


def kernel(x, edge_index, W1, b1, W2, b2):
    out, _ = run(FULL, dict(x=x, edge_index=edge_index, W1=W1, b1=b1,
                            W2=W2, b2=b2))
    return out



# revision 15
# speedup vs baseline: 1.0066x; 1.0066x over previous
"""GCN encoder (2-layer GCNConv) as a Bass/Tile kernel on 8 Trainium2 NeuronCores.

Strategy (matches the sharding hint):
  - Nodes row-partitioned across 8 cores (6250 rows each); weights replicated.
  - Symmetric normalization factorized: z = D^-1/2 (A+I) D^-1/2 (x W) + b
    =>  u = dinv * (x W);  agg[d] = u[d] + sum_{e:dst=d} u[src_e];
        z = dinv * agg + b
    so no per-edge norm gather is needed.
  - Per layer: local matmul -> row scale -> chunked AllGather of u (G row
    groups, each its own DRAM tensor so gathers can start after the first
    chunk lands) -> per-core gather of source rows (dma_gather) ->
    segment-sum via tensor-engine matmuls with 0/1 selection matrices
    generated on DVE (is_equal against an iota) -> scale/bias/relu.
  - Edges are bucketed host-side by (dst window of 128, src row-group) and
    padded to 128-slot tiles; padded slots gather row 0 and have an all-zero
    selection column, so they contribute nothing.  Chunk tensors hold
    8*ceil(6250/G) <= 8336 rows, so int16 gather indices always fit.
  - Aggregation runs in batches of B dst windows whose PSUM accumulators stay
    open across all G source groups; gathers are issued group-major so the
    gather stream only ever waits on the next AllGather chunk, which arrives
    while the previous chunk's gathers run.
  - Layer-2's u2 = dinv*(h1 @ W2) is computed per-window as soon as that
    window's layer-1 aggregation closes, and the layer-2 AllGather chunks
    fire mid-stream, so layer-2 gathers start right after layer-1's end.
"""

import math
import sys

import numpy as np

sys.path.insert(0, "/opt/trn_rl_repo")

import ml_dtypes

BF16 = ml_dtypes.bfloat16


class Cfg:
    def __init__(self, N, E, IN=512, HID=256, OUT=128, P=8, G=6, B=6):
        self.N, self.E, self.IN, self.HID, self.OUT, self.P = N, E, IN, HID, OUT, P
        self.NC = N // P                      # nodes per core
        self.WS = 128                         # dst window size
        self.NW = math.ceil(self.NC / self.WS)  # windows per core
        self.G = G                            # source row-groups (AG chunks)
        self.B = B                            # dst windows per PSUM batch
        self.R = math.ceil(self.NC / G)       # rows per group per core
        self.Rg = [min(self.NC, (g + 1) * self.R) - g * self.R for g in range(G)]
        assert P * self.R <= 32767            # int16 gather indices


FULL = Cfg(N=50000, E=800000)


def _prepare(cfg, x, edge_index, W1, b1, W2, b2):
    """Host-side graph preprocessing -> per-core input maps + program params."""
    N, P, NC, WS, NW = cfg.N, cfg.P, cfg.NC, cfg.WS, cfg.NW
    G, R = cfg.G, cfg.R
    src = np.asarray(edge_index[0], dtype=np.int64)
    dst = np.asarray(edge_index[1], dtype=np.int64)

    deg = np.bincount(dst, minlength=N).astype(np.float64) + 1.0  # + self loop
    dinv = (1.0 / np.sqrt(deg)).astype(np.float32)

    Rg = np.asarray(cfg.Rg, dtype=np.int64)
    rs = np.arange(G, dtype=np.int64) * R

    c_s = src // NC
    l_s = src % NC
    g_s = np.minimum(l_s // R, G - 1)
    row_in_chunk = c_s * Rg[g_s] + (l_s - rs[g_s])

    # group id: ((dst core, dst window), src row-group); contiguous after sort
    win_id = (dst // NC) * NW + (dst % NC) // WS
    comp = win_id * G + g_s
    order = np.argsort(comp, kind="stable")
    ric_s, d_s = row_in_chunk[order], dst[order]
    counts = np.bincount(comp[order], minlength=P * NW * G).reshape(P, NW, G)

    # shared tile counts per (window, group): max over cores
    T = np.ceil(counts.max(axis=0) / 128).astype(np.int64)  # [NW, G]
    tiles_total = int(T.sum())
    slots_total = tiles_total * 128

    starts = np.zeros(P * NW * G + 1, dtype=np.int64)
    np.cumsum(counts.reshape(-1), out=starts[1:])

    in_maps = []
    for c in range(P):
        idx_arr = np.zeros(slots_total, dtype=np.int16)   # pad: gather row 0
        aco_arr = np.full(slots_total, -1, dtype=np.float32)
        off = 0
        for w in range(NW):
            for g in range(G):
                grp = (c * NW + w) * G + g
                n = counts[c, w, g]
                sl = slice(starts[grp], starts[grp] + n)
                idx_arr[off:off + n] = ric_s[sl].astype(np.int16)
                aco_arr[off:off + n] = (d_s[sl] - c * NC - w * WS).astype(np.float32)
                off += 128 * int(T[w, g])
        assert off == slots_total

        dloc = np.concatenate(
            [dinv[c * NC:(c + 1) * NC],
             np.ones(NW * WS - NC, dtype=np.float32)])

        m = {
            "xT": np.ascontiguousarray(
                np.asarray(x[c * NC:(c + 1) * NC], np.float32).astype(BF16).T),
            "w1": np.ascontiguousarray(
                np.asarray(W1, np.float32).astype(BF16)
                .reshape(cfg.IN // 128, 128, cfg.HID).transpose(1, 0, 2)),
            "w2": np.ascontiguousarray(
                np.asarray(W2, np.float32).astype(BF16)
                .reshape(cfg.HID // 128, 128, cfg.OUT).transpose(1, 0, 2)),
            "dinvc": np.ascontiguousarray(dloc.reshape(NW, WS).T),
            "idx": np.ascontiguousarray(np.tile(idx_arr.reshape(-1, 16).T, (8, 1))),
            "acol": np.ascontiguousarray(aco_arr.reshape(-1, 128).T.astype(BF16)),
            "ident": np.eye(128, dtype=BF16),
        }
        b1nz = bool(np.any(np.asarray(b1)))
        b2nz = bool(np.any(np.asarray(b2)))
        if b1nz:
            m["b1bc"] = np.ascontiguousarray(
                np.broadcast_to(np.asarray(b1, np.float32), (128, cfg.HID)))
        if b2nz:
            m["b2bc"] = np.ascontiguousarray(
                np.broadcast_to(np.asarray(b2, np.float32), (128, cfg.OUT)))
        in_maps.append(m)

    return in_maps, T, b1nz, b2nz


def build_program(cfg, T, b1nz, b2nz):
    import concourse.bass as bass
    import concourse.bacc as bacc
    import concourse.mybir as mybir
    from concourse import tile

    N, P, NC, WS, NW = cfg.N, cfg.P, cfg.NC, cfg.WS, cfg.NW
    G, R, B = cfg.G, cfg.R, cfg.B
    Rg = cfg.Rg
    rs = [g * R for g in range(G)]
    IN, HID, OUT = cfg.IN, cfg.HID, cfg.OUT
    NCI, NCH = IN // 128, HID // 128
    tiles_total = int(T.sum())
    slots_total = tiles_total * 128
    f32, bf16, i16 = mybir.dt.float32, mybir.dt.bfloat16, mybir.dt.int16
    AF = mybir.ActivationFunctionType

    # host tables for the batched aggregation
    tile_start = np.zeros((NW, G), dtype=np.int64)   # running tile index
    acc = 0
    for w in range(NW):
        for g in range(G):
            tile_start[w, g] = acc
            acc += int(T[w, g])
    last_g = [-1] * NW
    for w in range(NW):
        for g in range(G):
            if T[w, g] > 0:
                last_g[w] = g
    # window whose close completes each row-group (for firing AG chunks)
    fire_w = [(rs[g] + Rg[g] - 1) // WS for g in range(G)]

    nc = bacc.Bacc("TRN2", target_bir_lowering=False, debug=False,
                   num_devices=cfg.P)
    xT_p = nc.dram_tensor("xT", [IN, NC], bf16, kind="ExternalInput")
    w1_p = nc.dram_tensor("w1", [128, NCI, HID], bf16, kind="ExternalInput")
    w2_p = nc.dram_tensor("w2", [128, NCH, OUT], bf16, kind="ExternalInput")
    dinv_p = nc.dram_tensor("dinvc", [WS, NW], f32, kind="ExternalInput")
    idx_p = nc.dram_tensor("idx", [128, slots_total // 16], i16, kind="ExternalInput")
    acol_p = nc.dram_tensor("acol", [128, tiles_total], bf16, kind="ExternalInput")
    id_p = nc.dram_tensor("ident", [128, 128], bf16, kind="ExternalInput")
    b1_p = (nc.dram_tensor("b1bc", [128, HID], f32, kind="ExternalInput")
            if b1nz else None)
    b2_p = (nc.dram_tensor("b2bc", [128, OUT], f32, kind="ExternalInput")
            if b2nz else None)
    out_p = nc.dram_tensor("out", [NC, OUT], f32, kind="ExternalOutput")

    # per-chunk local shards and AllGather outputs (separate tensors so the
    # Tile dependency from gathers to "their" chunk is precise)
    u1dc = [nc.dram_tensor(f"u1dc{g}", [Rg[g], HID], bf16) for g in range(G)]
    u2dc = [nc.dram_tensor(f"u2dc{g}", [Rg[g], OUT], bf16) for g in range(G)]
    U1c = [nc.dram_tensor(f"U1c{g}", [P * Rg[g], HID], bf16) for g in range(G)]
    U2c = [nc.dram_tensor(f"U2c{g}", [P * Rg[g], OUT], bf16) for g in range(G)]
    rg_all = [list(range(P))]

    def nsz(j):
        return min(128, NC - j * WS)

    def write_chunked(dsts, w, n, src_tile):
        """Write rows [w*WS, w*WS+n) of a window tile into the per-group
        chunk tensors, splitting at group boundaries."""
        a = w * WS
        b = a + n
        for g in range(G):
            ga, gb = rs[g], rs[g] + Rg[g]
            lo, hi = max(a, ga), min(b, gb)
            if lo < hi:
                nc.sync.dma_start(dsts[g][lo - ga:hi - ga, :],
                                  src_tile[lo - a:hi - a, :])

    with tile.TileContext(nc) as tc:
        with (
            tc.tile_pool(name="res", bufs=1) as res,
            tc.tile_pool(name="work", bufs=4) as work,
            tc.tile_pool(name="psum", bufs=1, space="PSUM") as psum,
        ):
            # ---- resident loads ----
            xTs = res.tile([128, NCI, NC], bf16)
            for ci in range(NCI):
                nc.sync.dma_start(xTs[:, ci, :], xT_p[ci * 128:(ci + 1) * 128, :])
            w1s = res.tile([128, NCI, HID], bf16)
            nc.sync.dma_start(w1s[:], w1_p[:])
            w2s = res.tile([128, NCH, OUT], bf16)
            nc.sync.dma_start(w2s[:], w2_p[:])
            dinvs = res.tile([WS, NW], f32)
            nc.sync.dma_start(dinvs[:], dinv_p[:])
            idxs = res.tile([128, slots_total // 16], i16)
            nc.sync.dma_start(idxs[:], idx_p[:])
            acols = res.tile([128, tiles_total], bf16)
            nc.sync.dma_start(acols[:], acol_p[:])
            ident = res.tile([128, 128], bf16)
            nc.sync.dma_start(ident[:], id_p[:])
            iot = res.tile([128, 128], bf16)
            nc.gpsimd.iota(iot[:], pattern=[[1, 128]], base=0,
                           channel_multiplier=0,
                           allow_small_or_imprecise_dtypes=True)
            b1bc = None
            if b1nz:
                b1bc = res.tile([128, HID], f32)
                nc.sync.dma_start(b1bc[:], b1_p[:])
            b2bc = None
            if b2nz:
                b2bc = res.tile([128, OUT], f32)
                nc.sync.dma_start(b2bc[:], b2_p[:])

            # manually-rotated gather staging buffer
            gmax = int(T.max())
            GB = 4
            gbuf = res.tile([128, GB * gmax * HID], bf16)
            nc.gpsimd.memset(gbuf[:], 0.0)
            gslot = [0]

            u1res = res.tile([128, NW, HID], bf16)
            u2res = res.tile([128, NW, OUT], bf16)
            h1T = res.tile([128, NCH, NC], bf16)
            if NC % WS:
                # tail rows of the last window feed the self-loop matmul as
                # rhs; zero them so uninitialized SBUF can't inject NaNs
                nc.gpsimd.memset(u1res[:, NW - 1, :], 0.0)
                nc.gpsimd.memset(u2res[:, NW - 1, :], 0.0)

            # ---- phase A: t1 = x @ W1 ; u1 = dinv * t1 ; chunked AllGather ----
            for j in range(NW):
                n = nsz(j)
                jsl = slice(j * WS, j * WS + n)
                pt = psum.tile([128, HID], f32, tag="mm", bufs=1)
                for ci in range(NCI):
                    nc.tensor.matmul(pt[:n, :], xTs[:, ci, jsl],
                                     w1s[:, ci, :], start=(ci == 0),
                                     stop=(ci == NCI - 1))
                nc.scalar.activation(u1res[:n, j, :], pt[:n, :], AF.Copy,
                                     scale=dinvs[:n, j:j + 1])
                write_chunked(u1dc, j, n, u1res[:, j, :])
                for g in range(G):
                    if fire_w[g] == j:
                        nc.gpsimd.collective_compute(
                            "AllGather", mybir.AluOpType.bypass,
                            replica_groups=rg_all,
                            ins=[u1dc[g][:]], outs=[U1c[g][:]])

            # ---- batched aggregation layer ----
            def agg_layer(Uc, F, ures, bbc, relu, emit_out):
                for b0 in range(0, NW, B):
                    batch = list(range(b0, min(b0 + B, NW)))
                    pa = {}
                    for w in batch:
                        n = nsz(w)
                        pa[w] = psum.tile([128, F], f32, tag="agg", bufs=B,
                                          name=f"pa{w}")
                        # self-loop term: ident.T @ u[w]
                        nc.tensor.matmul(pa[w][:n, :], ident[:, :n],
                                         ures[:, w, :], start=True,
                                         stop=(last_g[w] < 0))
                    for g in range(G):
                        for w in batch:
                            t_wg = int(T[w, g])
                            if t_wg == 0:
                                continue
                            n = nsz(w)
                            ts0 = int(tile_start[w, g])
                            goff = (gslot[0] % GB) * gmax * HID
                            gslot[0] += 1
                            gv = gbuf[:, goff:goff + t_wg * F].rearrange(
                                "p (t f) -> p t f", t=t_wg)
                            nc.gpsimd.dma_gather(
                                gv[:], Uc[g][:],
                                idxs[:, ts0 * 8:(ts0 + t_wg) * 8],
                                num_idxs=128 * t_wg,
                                num_idxs_reg=128 * t_wg,
                                elem_size=F, single_packet=False)
                            for t in range(t_wg):
                                S = work.tile([128, 128], bf16, tag="S")
                                nc.vector.tensor_tensor(
                                    S[:], iot[:],
                                    acols[:, ts0 + t:ts0 + t + 1]
                                    .broadcast_to((128, 128)),
                                    op=mybir.AluOpType.is_equal)
                                nc.tensor.matmul(
                                    pa[w][:n, :], S[:, :n], gv[:, t, :],
                                    start=False,
                                    stop=(g == last_g[w] and t == t_wg - 1))
                    for w in batch:
                        n = nsz(w)
                        # z = dinv * agg (+ b) ; relu
                        if bbc is None:
                            zf = AF.Relu if relu else AF.Copy
                            zt = work.tile([128, F], f32 if emit_out else bf16,
                                           tag="zt%d" % F)
                            nc.scalar.activation(zt[:n, :], pa[w][:n, :], zf,
                                                 scale=dinvs[:n, w:w + 1])
                        else:
                            v = work.tile([128, F], f32, tag="v%d" % F)
                            nc.scalar.activation(v[:n, :], pa[w][:n, :],
                                                 AF.Copy,
                                                 scale=dinvs[:n, w:w + 1])
                            zt = work.tile([128, F], f32 if emit_out else bf16,
                                           tag="zt%d" % F)
                            if relu:
                                vb = work.tile([128, F], f32, tag="vb%d" % F)
                                nc.vector.tensor_tensor(
                                    vb[:n, :], v[:n, :], bbc[:n, :],
                                    op=mybir.AluOpType.add)
                                nc.scalar.activation(zt[:n, :], vb[:n, :],
                                                     AF.Relu)
                            else:
                                nc.vector.tensor_tensor(
                                    zt[:n, :], v[:n, :], bbc[:n, :],
                                    op=mybir.AluOpType.add)
                        yield w, n, zt

            # ---- layer-1 aggregation; phase D inline per window ----
            for w, n, zt in agg_layer(U1c, HID, u1res, b1bc, True, False):
                wsl = slice(w * WS, w * WS + n)
                for ch in range(NCH):
                    ptr = psum.tile([128, 128], bf16, tag="tr", bufs=1)
                    nc.tensor.transpose(ptr[:, :n],
                                        zt[:n, ch * 128:(ch + 1) * 128],
                                        ident[:n, :n])
                    nc.scalar.activation(h1T[:, ch, wsl], ptr[:, :n], AF.Copy)
                # t2 = h1 @ W2 ; u2 = dinv * t2
                pt = psum.tile([128, OUT], f32, tag="mm", bufs=1)
                for ch in range(NCH):
                    nc.tensor.matmul(pt[:n, :], h1T[:, ch, wsl],
                                     w2s[:, ch, :], start=(ch == 0),
                                     stop=(ch == NCH - 1))
                nc.scalar.activation(u2res[:n, w, :], pt[:n, :], AF.Copy,
                                     scale=dinvs[:n, w:w + 1])
                write_chunked(u2dc, w, n, u2res[:, w, :])
                for g in range(G):
                    if fire_w[g] == w:
                        nc.gpsimd.collective_compute(
                            "AllGather", mybir.AluOpType.bypass,
                            replica_groups=rg_all,
                            ins=[u2dc[g][:]], outs=[U2c[g][:]])

            # ---- layer-2 aggregation -> out ----
            for w, n, zt in agg_layer(U2c, OUT, u2res, b2bc, False, True):
                wsl = slice(w * WS, w * WS + n)
                nc.sync.dma_start(out_p[wsl, :], zt[:n, :])

    return nc


def run(cfg, inputs, sim=False, trace=False):
    from concourse.bass_utils import run_bass_kernel_spmd

    in_maps, T, b1nz, b2nz = _prepare(
        cfg, inputs["x"], inputs["edge_index"], inputs["W1"], inputs["b1"],
        inputs["W2"], inputs["b2"])
    nc = build_program(cfg, T, b1nz, b2nz)
    nc.finalize()
    core_ids = list(range(cfg.P))
    if sim:
        from concourse import bass_interp
        ms = bass_interp.MultiCoreSim(nc, cfg.P)
        for c in core_ids:
            for k, v in in_maps[c].items():
                ms.cores[c].tensor(k)[:] = v
        ms.simulate()
        outs = [np.array(ms.cores[c].tensor("out")) for c in core_ids]
        return np.concatenate(outs, axis=0), None
    res = run_bass_kernel_spmd(nc, in_maps, core_ids, trace=trace)
    outs = [np.asarray(res.results[c]["out"]) for c in core_ids]
    return np.concatenate(outs, axis=0), res


def kernel(x, edge_index, W1, b1, W2, b2):
    out, _ = run(FULL, dict(x=x, edge_index=edge_index, W1=W1, b1=b1,
                            W2=W2, b2=b2))
    return out
